# revision 1
# baseline (speedup 1.0000x reference)
"""GCN layer (gnn_message_passing) Trainium2 Bass kernel.

Problem: out[b,n,:] = relu( sum_r (mean_k padded[b, idx[b,r,n,k]]) @ W_r
                            + feat[b,n] @ W_self + bias )
  B=4, N=4096, D=O=128, R=4, K=16.

Strategy: shard (batch x N-half) across 8 cores -> no collectives.
Per core (b, h):
  - DRAM table tbl[4097, 128] bf16 = [zeros; node_features[b]] (host-cast).
  - SWDGE dma_gather (transpose=True) pulls neighbor rows as columns
    [d, j] in bf16; relation r's stream is idx[b,r,n,k] in natural order
    (k innermost), the "self" stream is n+1.
  - DVE tensor_reduce sums k (innermost 16) -> aggT_r [d, n] f32.
  - PE: out_psum[n, o] = sum_r aggT_r_slice.T @ (W_r/K) + selfT.T @ W_self
        + ones.T @ bias  (f32 matmuls, accumulated in PSUM).
  - ACT applies ReLU, HWDGE stores [n, o] f32 rows.
"""

import numpy as np
import ml_dtypes

import concourse.bacc as bacc
import concourse.mybir as mybir
from concourse.tile import TileContext
from concourse.bass_utils import run_bass_kernel_spmd

B, N, D = 4, 4096, 128
R, K, O = 4, 16, 128
NCORES = 8
NH = N // 2            # nodes per core
CHUNK = 512            # nodes per chunk
NCH = NH // CHUNK      # chunks per core
RJ = CHUNK * K         # idxs per relation-call (8192)
SEG = R * RJ // 16 + CHUNK // 16   # idx cols per chunk: 4*512 + 32 = 2080
G_BUFS = 6

_cache = {}


def _build():
    nc = bacc.Bacc("TRN2")
    tbl = nc.dram_tensor("tbl", [N + 1, D], mybir.dt.bfloat16, kind="ExternalInput")
    idxs = nc.dram_tensor("idxs", [128, NCH * SEG], mybir.dt.int16, kind="ExternalInput")
    w = nc.dram_tensor("w", [128, R + 2, O], mybir.dt.float32, kind="ExternalInput")
    out = nc.dram_tensor("out", [NH, O], mybir.dt.float32, kind="ExternalOutput")

    with TileContext(nc) as tc:
        with (
            tc.tile_pool(name="const", bufs=1) as cpool,
            tc.tile_pool(name="idx", bufs=2) as ipool,
            tc.tile_pool(name="g", bufs=G_BUFS) as gpool,
            tc.tile_pool(name="gs", bufs=2) as gspool,
            tc.tile_pool(name="agg", bufs=6) as apool,
            tc.tile_pool(name="aggs", bufs=2) as aspool,
            tc.tile_pool(name="osb", bufs=2) as opool,
            tc.tile_pool(name="ps", bufs=8, space="PSUM") as pspool,
        ):
            w_sb = cpool.tile([128, R + 2, O], mybir.dt.float32)
            nc.sync.dma_start(w_sb[:], w[:])
            ones = cpool.tile([1, 128], mybir.dt.float32)
            nc.vector.memset(ones[:], 1.0)

            for ch in range(NCH):
                idx_sb = ipool.tile([128, SEG], mybir.dt.int16)
                nc.sync.dma_start(idx_sb[:], idxs[:, ch * SEG:(ch + 1) * SEG])

                aggs = []
                for r in range(R):
                    g = gpool.tile([128, 1, RJ], mybir.dt.bfloat16, tag="g")
                    nc.gpsimd.dma_gather(
                        g[:], tbl[:],
                        idx_sb[:, r * (RJ // 16):(r + 1) * (RJ // 16)],
                        RJ, RJ, D, transpose=True, single_packet=False,
                    )
                    aggf = apool.tile([128, CHUNK], mybir.dt.float32, tag="aggf")
                    nc.vector.tensor_reduce(
                        aggf[:],
                        g[:, 0, :].rearrange("p (n k) -> p n k", k=K),
                        mybir.AxisListType.X,
                        mybir.AluOpType.add,
                    )
                    aggs.append(aggf)

                g_s = gspool.tile([128, 1, CHUNK], mybir.dt.bfloat16, tag="gs")
                nc.gpsimd.dma_gather(
                    g_s[:], tbl[:],
                    idx_sb[:, R * (RJ // 16):],
                    CHUNK, CHUNK, D, transpose=True, single_packet=False,
                )
                agg_s = aspool.tile([128, CHUNK], mybir.dt.float32, tag="aggsf")
                nc.vector.tensor_copy(agg_s[:], g_s[:, 0, :])

                out_sb = opool.tile([128, CHUNK // 128, O], mybir.dt.float32)
                for t in range(CHUNK // 128):
                    ps = pspool.tile([128, O], mybir.dt.float32)
                    sl = slice(t * 128, (t + 1) * 128)
                    for r in range(R):
                        nc.tensor.matmul(
                            ps[:], aggs[r][:, sl], w_sb[:, r, :],
                            start=(r == 0), stop=False,
                        )
                    nc.tensor.matmul(
                        ps[:], agg_s[:, sl], w_sb[:, R, :],
                        start=False, stop=False,
                    )
                    nc.tensor.matmul(
                        ps[:], ones[:1, :], w_sb[0:1, R + 1, :],
                        start=False, stop=True,
                    )
                    nc.scalar.activation(
                        out_sb[:, t, :], ps[:], mybir.ActivationFunctionType.Relu
                    )
                nc.sync.dma_start(
                    out[ch * CHUNK:(ch + 1) * CHUNK, :].rearrange(
                        "(t p) o -> p t o", p=128
                    ),
                    out_sb[:],
                )

    nc.compile()
    return nc


def _prep_inputs(node_features, neighbor_indices, relation_kernels, self_kernel, bias):
    """Host-side shard/layout prep. Returns per-core input maps."""
    nf = np.asarray(node_features)
    idx = np.asarray(neighbor_indices)
    in_maps = []
    tbls = []
    for b in range(B):
        t = np.zeros((N + 1, D), dtype=ml_dtypes.bfloat16)
        t[1:] = nf[b].astype(ml_dtypes.bfloat16)
        tbls.append(t)

    w = np.zeros((128, R + 2, O), dtype=np.float32)
    for r in range(R):
        w[:, r, :] = np.asarray(relation_kernels)[r] / K
    w[:, R, :] = np.asarray(self_kernel)
    w[0, R + 1, :] = np.asarray(bias)

    for c in range(NCORES):
        b, h = divmod(c, 2)
        base = h * NH
        cols = np.empty((16, NCH * SEG), dtype=np.int16)
        for ch in range(NCH):
            seg = np.empty((16, SEG), dtype=np.int16)
            for r in range(R):
                stream = idx[b, r, base + ch * CHUNK: base + (ch + 1) * CHUNK, :]
                stream = stream.reshape(-1).astype(np.int16)
                seg[:, r * (RJ // 16):(r + 1) * (RJ // 16)] = stream.reshape(-1, 16).T
            selfs = np.arange(base + ch * CHUNK + 1, base + (ch + 1) * CHUNK + 1,
                              dtype=np.int16)
            seg[:, R * (RJ // 16):] = selfs.reshape(-1, 16).T
            cols[:, ch * SEG:(ch + 1) * SEG] = seg
        in_maps.append({
            "tbl": tbls[b],
            "idxs": np.tile(cols, (8, 1)),
            "w": w,
        })
    return in_maps


def _run(in_maps, **kw):
    if "nc" not in _cache:
        _cache["nc"] = _build()
    return run_bass_kernel_spmd(_cache["nc"], in_maps, core_ids=list(range(NCORES)), **kw)


def kernel(node_features, neighbor_indices, relation_kernels, self_kernel, bias):
    in_maps = _prep_inputs(node_features, neighbor_indices, relation_kernels,
                           self_kernel, bias)
    res = _run(in_maps)
    out = np.empty((B, N, O), dtype=np.float32)
    for c in range(NCORES):
        b, h = divmod(c, 2)
        out[b, h * NH:(h + 1) * NH, :] = res.results[c]["out"]
    return out



# revision 4
# speedup vs baseline: 2.7013x; 2.7013x over previous
"""GCN layer (gnn_message_passing) Trainium2 Bass kernel.

Problem: out[b,n,:] = relu( sum_r (mean_k padded[b, idx[b,r,n,k]]) @ W_r
                            + feat[b,n] @ W_self + bias )
  B=4, N=4096, D=O=128, R=4, K=16.

Strategy: shard (batch x N-half) across 8 cores -> no collectives.

Per core (b, h), project-then-gather:
  Prologue (PE):
    ptbl_r = padded @ (W_r/K)  [4097p, O] bf16 -- one ldweights per
    128-row tile of paddedT, one matmul streaming all 4 relations'
    kernels (512 cols) into PSUM; DVE copies to SBUF bf16; HWDGE
    writes each relation's table back to DRAM.
    selfm = feat @ W_self + bias  [NH, O] bf16 kept in SBUF.
  Main loop, per (chunk of 512 nodes, relation):
    - SWDGE dma_gather (transpose=False -> plain copy descriptors, safe
      to run concurrently) pulls projected rows [p, c, o]; queue_num=r
      puts each relation's descriptor generation on its own Q7 core
      pair, 4-way parallel. Stream order is k-outer (j = k*512 + n) so
      node n's K rows share partition n%128.
    - DVE tensor_reduce over k (stride 4*256B) -> msg_r [n%128, s, o].
    - DVE adds the 4 relations + selfm; ACT applies ReLU -> f32 out.
  (Transpose-mode gathers are NOT safe on multiple queues: their rows
  funnel through shared per-SDMA-engine XBAR staging, and interleaved
  packets from different queues corrupt the 16-row transpose groups.)
"""

import numpy as np
import ml_dtypes

import concourse.bacc as bacc
import concourse.mybir as mybir
from concourse.tile import TileContext
from concourse.bass_utils import run_bass_kernel_spmd

B, N, D = 4, 4096, 128
R, K, O = 4, 16, 128
NCORES = 8
NH = N // 2            # nodes per core
CHUNK = 512            # nodes per chunk
NCH = NH // CHUNK      # chunks per core
RJ = CHUNK * K         # idxs per relation-call (8192)
SEG = R * RJ // 16     # idx cols per chunk: 4*512 = 2048
NT = 33                # 128-row tiles covering the 4097-row table
TROWS = NT * 128       # padded table rows (4224)
G_BUFS = 6

_cache = {}


def _build():
    nc = bacc.Bacc("TRN2", num_swdge_queues=4)
    tblT = nc.dram_tensor("tblT", [128, TROWS], mybir.dt.bfloat16, kind="ExternalInput")
    idxs = nc.dram_tensor("idxs", [128, NCH * SEG], mybir.dt.int16, kind="ExternalInput")
    w = nc.dram_tensor("w", [128, R + 2, O], mybir.dt.bfloat16, kind="ExternalInput")
    featT = nc.dram_tensor("featT", [128, NH], mybir.dt.bfloat16, kind="ExternalInput")
    out = nc.dram_tensor("out", [NH, O], mybir.dt.float32, kind="ExternalOutput")

    with TileContext(nc) as tc:
        with (
            tc.tile_pool(name="const", bufs=1) as cpool,
            tc.tile_pool(name="ptd", bufs=1, space="DRAM") as dpool,
            tc.tile_pool(name="idx", bufs=2) as ipool,
            tc.tile_pool(name="g", bufs=G_BUFS) as gpool,
            tc.tile_pool(name="agg", bufs=6) as apool,
            tc.tile_pool(name="sum", bufs=4) as spool,
            tc.tile_pool(name="osb", bufs=2) as opool,
            tc.tile_pool(name="pp", bufs=4, space="PSUM") as prpool,
            tc.tile_pool(name="ps", bufs=2, space="PSUM") as pspool,
        ):
            w_sb = cpool.tile([128, R + 2, O], mybir.dt.bfloat16)
            nc.sync.dma_start(w_sb[:], w[:])
            tblT_sb = cpool.tile([128, TROWS], mybir.dt.bfloat16)
            nc.sync.dma_start(tblT_sb[:], tblT[:])
            feat_sb = cpool.tile([128, NH], mybir.dt.bfloat16)
            nc.sync.dma_start(feat_sb[:], featT[:])
            ones = cpool.tile([1, 128], mybir.dt.bfloat16)
            nc.vector.memset(ones[:], 1.0)

            # --- prologue: project the table through all relation kernels ---
            ptbl_sb = cpool.tile([128, R, NT, O], mybir.dt.bfloat16)
            for t in range(NT):
                pp = prpool.tile([128, R, O], mybir.dt.float32)
                nc.tensor.matmul(
                    pp[:], tblT_sb[:, t * 128:(t + 1) * 128], w_sb[:, 0:R, :],
                    start=True, stop=True,
                )
                with nc.allow_low_precision(reason="bf16 gather table"):
                    for r in range(R):
                        nc.vector.tensor_copy(ptbl_sb[:, r, t, :], pp[:, r, :])
            ptbls = []
            for r in range(R):
                pt = dpool.tile([TROWS, O], mybir.dt.bfloat16, tag=f"pt{r}")
                nc.sync.dma_start(
                    pt[:].rearrange("(t p) o -> p t o", p=128), ptbl_sb[:, r, :, :]
                )
                ptbls.append(pt)

            # --- prologue: self messages (feat @ W_self + bias) ---
            selfm_sb = cpool.tile([128, NH // 128, O], mybir.dt.bfloat16)
            for t in range(NH // 128):
                ps = pspool.tile([128, O], mybir.dt.float32, tag="selfps")
                nc.tensor.matmul(
                    ps[:], feat_sb[:, t * 128:(t + 1) * 128], w_sb[:, R, :],
                    start=True, stop=False,
                )
                nc.tensor.matmul(
                    ps[:], ones[:1, :], w_sb[0:1, R + 1, :],
                    start=False, stop=True,
                )
                with nc.allow_low_precision(reason="bf16 self msg"):
                    nc.scalar.activation(
                        selfm_sb[:, t, :], ps[:],
                        mybir.ActivationFunctionType.Copy,
                    )

            # --- main loop ---
            for ch in range(NCH):
                idx_sb = ipool.tile([128, SEG], mybir.dt.int16)
                nc.sync.dma_start(idx_sb[:], idxs[:, ch * SEG:(ch + 1) * SEG])

                aggs = []
                for r in range(R):
                    g = gpool.tile([128, RJ // 128, D], mybir.dt.bfloat16, tag="g")
                    nc.gpsimd.dma_gather(
                        g[:], ptbls[r][:],
                        idx_sb[:, r * (RJ // 16):(r + 1) * (RJ // 16)],
                        RJ, RJ, D, transpose=False, single_packet=False,
                        queue_num=r,
                    )
                    # stream j = k*512 + n: row j at [p=j%128, c=j//128] with
                    # c = k*4 + s, node n = s*128 + p. Reduce over k.
                    aggf = apool.tile([128, CHUNK // 128, O], mybir.dt.bfloat16,
                                      tag="aggf")
                    with nc.allow_low_precision(reason="bf16 msg sums"):
                        nc.vector.tensor_reduce(
                            aggf[:],
                            g[:].rearrange("p (k s) e -> p s e k", k=K),
                            mybir.AxisListType.X,
                            mybir.AluOpType.add,
                        )
                    aggs.append(aggf)

                with nc.allow_low_precision(reason="bf16 msg sums"):
                    s01 = spool.tile([128, CHUNK // 128, O], mybir.dt.bfloat16,
                                     tag="s01")
                    nc.vector.tensor_tensor(
                        s01[:], aggs[0][:], aggs[1][:], mybir.AluOpType.add
                    )
                    s23 = spool.tile([128, CHUNK // 128, O], mybir.dt.bfloat16,
                                     tag="s23")
                    nc.vector.tensor_tensor(
                        s23[:], aggs[2][:], aggs[3][:], mybir.AluOpType.add
                    )
                    stot = spool.tile([128, CHUNK // 128, O], mybir.dt.bfloat16,
                                      tag="stot")
                    nc.vector.tensor_tensor(
                        stot[:], s01[:], s23[:], mybir.AluOpType.add
                    )
                    sall = spool.tile([128, CHUNK // 128, O], mybir.dt.bfloat16,
                                      tag="sall")
                    nc.vector.tensor_tensor(
                        sall[:],
                        stot[:],
                        selfm_sb[:, ch * (CHUNK // 128):(ch + 1) * (CHUNK // 128), :],
                        mybir.AluOpType.add,
                    )
                out_sb = opool.tile([128, CHUNK // 128, O], mybir.dt.float32)
                nc.scalar.activation(
                    out_sb[:], sall[:], mybir.ActivationFunctionType.Relu
                )
                nc.sync.dma_start(
                    out[ch * CHUNK:(ch + 1) * CHUNK, :].rearrange(
                        "(t p) o -> p t o", p=128
                    ),
                    out_sb[:],
                )

    nc.compile()
    return nc


def _prep_inputs(node_features, neighbor_indices, relation_kernels, self_kernel, bias):
    """Host-side shard/layout prep. Returns per-core input maps."""
    nf = np.asarray(node_features)
    idx = np.asarray(neighbor_indices)
    in_maps = []
    tblTs = []
    for b in range(B):
        t = np.zeros((128, TROWS), dtype=ml_dtypes.bfloat16)
        t[:, 1:N + 1] = nf[b].astype(ml_dtypes.bfloat16).T
        tblTs.append(t)

    w = np.zeros((128, R + 2, O), dtype=ml_dtypes.bfloat16)
    for r in range(R):
        w[:, r, :] = (np.asarray(relation_kernels)[r] / K).astype(ml_dtypes.bfloat16)
    w[:, R, :] = np.asarray(self_kernel).astype(ml_dtypes.bfloat16)
    w[0, R + 1, :] = np.asarray(bias).astype(ml_dtypes.bfloat16)

    for c in range(NCORES):
        b, h = divmod(c, 2)
        base = h * NH
        cols = np.empty((16, NCH * SEG), dtype=np.int16)
        for ch in range(NCH):
            seg = np.empty((16, SEG), dtype=np.int16)
            for r in range(R):
                stream = idx[b, r, base + ch * CHUNK: base + (ch + 1) * CHUNK, :]
                # k-outer stream order: j = k*CHUNK + n
                stream = stream.T.reshape(-1).astype(np.int16)
                seg[:, r * (RJ // 16):(r + 1) * (RJ // 16)] = stream.reshape(-1, 16).T
            cols[:, ch * SEG:(ch + 1) * SEG] = seg
        featT = np.ascontiguousarray(
            nf[b, base:base + NH, :].astype(ml_dtypes.bfloat16).T
        )
        in_maps.append({
            "tblT": tblTs[b],
            "idxs": np.tile(cols, (8, 1)),
            "w": w,
            "featT": featT,
        })
    return in_maps


def _run(in_maps, **kw):
    if "nc" not in _cache:
        _cache["nc"] = _build()
    return run_bass_kernel_spmd(_cache["nc"], in_maps, core_ids=list(range(NCORES)), **kw)


def kernel(node_features, neighbor_indices, relation_kernels, self_kernel, bias):
    in_maps = _prep_inputs(node_features, neighbor_indices, relation_kernels,
                           self_kernel, bias)
    res = _run(in_maps)
    out = np.empty((B, N, O), dtype=np.float32)
    for c in range(NCORES):
        b, h = divmod(c, 2)
        out[b, h * NH:(h + 1) * NH, :] = res.results[c]["out"]
    return out
